# revision 31
# baseline (speedup 1.0000x reference)
"""TRN2 Bass/Tile kernel: Llama attention block (B=1, S=2048, D=2048, H=16, causal).

Sharding: tensor-parallel over heads. 16 heads / 8 cores = 2 heads per core.
Wq/Wk/Wv column-sharded, Wo applied to this core's 256 seq rows after a
per-head AllToAll of attn.T (sequence-parallel output projection).

Per-core dataflow (bf16 matmul operands, fp32 PSUM accumulate):
  - host passes X.T and head-major wq/wk ([HPC,P,KT,HD]); DMA priority
    order: 2-ktile head-0 weight prefix, xt0, rest of the head-0 halves,
    the xt stream, then wv / rope tables / head-1 halves / masks in
    consumer order (all transfers serialize on the shared DMA engines)
  - RoPE rotate-half is a pure SBUF partition-swap DMA; the sign lives in
    the host sin table; q' = q*cos + rot(q)*sin' on GpSimd/DVE (NCC_IBIR297
    forbids a partition-offset two-SBUF-operand DVE read, so the swap
    cannot ride the vector engine)
  - phase order finishes head 0 as early as possible: qk(0) -> v-proj ->
    attention(h0) -> AllToAll#0, hidden under a software-pipelined head-1
    chain (per-block qk(1) -> rope -> attention); only AllToAll#1 is exposed
  - consecutive matmuls never target the same PSUM bank (a same-bank
    back-to-back pair stalls the real PE for ~1.7us, unmodeled by CoreSim):
    v-proj runs m-tile pairs on two banks, qk1 interleaves the wq/wk
    accumulations, the Wo passes rotate bank pairs ki-inner
  - attention: scoresT[t,sq] = k_tile.T @ qT (stationary k), exp on ScalarE
    (no max-subtraction: |scaled scores| < 5 here), causal mask by a 0/1
    staircase multiply, probs consumed as the moving operand of the
    v-matmul -> attn.T with zero transposes; diagonal tiles are narrowed to
    their unmasked columns in every stage
  - softmax denominators ride the PE: an all-ones stationary operand
    accumulates per-tile partition sums; full tiles are DVE-summed in
    quads first; on a block's last tile the denominator matmul precedes
    the pa matmul so the reciprocal hides under it; DVE reciprocal * attn
    must happen here (per-row after Wo would mix heads)
  - exchange staging and the post-collective gathers are single permuted
    DMAs per head (AP rearrange "j p s -> p j s")
  - Wo runs as two 8-bank PSUM passes (head-0 k-tiles after AllToAll#0,
    head-1 after AllToAll#1) anchored behind the attention stream in the PE
    queue; out copies/stores are pipelined per 512-column chunk
  - do NOT remove or batch the collectives: measured on HW, replacing the
    AllToAlls with local copies (or nothing) is net SLOWER - they pace the
    8 cores' shared-HBM phases into lockstep

A direct SBUF->SBUF remote-DMA exchange (BASS_RDMA=1) replaces both
AllToAlls with XOR-relative peer writes and runtime topology discovery;
it simulates ~14% faster but the current fleet's runtime crashes on
REMOTE_DMA_* instructions, so it stays off by default.
"""

import os
import sys

import numpy as np

for _p in ("/opt/trn_rl_repo",):
    if _p not in sys.path and os.path.isdir(_p):
        sys.path.insert(0, _p)

P = 128            # SBUF partitions
S = 2048           # sequence length
D = 2048           # hidden dim
NCORES = 8
DC = D // NCORES   # 256 = head-dims per core
HPC = 2            # heads per core
HD = 128           # head dim
KT = D // P        # 16 contraction tiles
SQW = 512          # sq tile width (moving free dim)
NSQ = S // SQW     # 4
NT = S // P        # 16 t tiles
SCS = S // NCORES  # 256 output seq rows per core (sequence-parallel Wo)
SM = float(1.0 / np.sqrt(HD))

# Cross-core exchange flavor: direct SBUF->SBUF remote DMA (fast, but needs
# runtime/ucode support the current fleet lacks — crashes the exec unit) vs
# the collectives AllToAll. Keep the rdma path selectable for future use.
RDMA = os.environ.get("BASS_RDMA", "0") == "1"

_NC_CACHE = {}
LAST_RESULTS = None


def _build_nc(reps=1):
    import concourse.bacc as bacc
    import concourse.mybir as mybir
    from concourse import tile

    import concourse.bass as bass

    fp32 = mybir.dt.float32
    bf16 = mybir.dt.bfloat16
    u32 = mybir.dt.uint32
    Exp = mybir.ActivationFunctionType.Exp

    nc = bacc.Bacc("TRN2", num_devices=NCORES, debug=False)

    # Cross-core exchange machinery: all peer traffic uses XOR-relative
    # remote_dma_broadcast (single live destination per send), so no physical
    # routing/core ids are ever needed. Slot identities are discovered at
    # runtime: every core broadcasts its logical id; XOR symmetry lands the
    # id of each core's delta-neighbor in receive slot delta on both sides.
    nsem = nc.alloc_semaphore("nbr_sem")    # prelude id-exchange arrivals
    rsem = [nc.alloc_semaphore(f"attn_sem{h}") for h in range(HPC)]
    lsem = nc.alloc_semaphore("rdma_lsem")  # local send completions (unused)

    def rdest(delta):
        return [(0, k) if k == delta else None for k in range(NCORES)]

    # (instruction, sem, threshold): cross-core arrival waits appended to
    # sync_info after tile lowering — the tile scheduling pass can't model
    # remote increments and would deadlock on an in-graph wait instruction
    post_waits = []

    myid = nc.dram_tensor("myid", [P, 1], u32, kind="ExternalInput")
    xt = nc.dram_tensor("xt", [D, S], bf16, kind="ExternalInput")
    # wq/wk arrive head-major [HPC, P, KT, HD] so the head-0 half (all the
    # qk0 phase needs) streams as its own 512KB with 4KB-contiguous rows
    wq = nc.dram_tensor("wq", [HPC, P, KT, HD], bf16, kind="ExternalInput")
    wk = nc.dram_tensor("wk", [HPC, P, KT, HD], bf16, kind="ExternalInput")
    wv = nc.dram_tensor("wv", [D, DC], bf16, kind="ExternalInput")
    wo = nc.dram_tensor("wo", [D, D], bf16, kind="ExternalInput")  # full Wo.T
    cost = nc.dram_tensor("cost", [HD, S], bf16, kind="ExternalInput")
    sint = nc.dram_tensor("sint", [HD, S], bf16, kind="ExternalInput")
    msk = nc.dram_tensor("msk", [P, 896], bf16, kind="ExternalInput")
    ones = nc.dram_tensor("ones", [P, P], bf16, kind="ExternalInput")
    out = nc.dram_tensor("out", [SCS, D], fp32, kind="ExternalOutput")

    xt_r = xt.rearrange("(k p) s -> p k s", p=P)
    wv_r = wv.rearrange("(k p) d -> p k d", p=P)
    wo_r = wo.rearrange("(k p) d -> p k d", p=P)
    out_r = out.rearrange("(m p) d -> m p d", p=P)

    def emit_body(tc, rep, const, consts):
        r = f"r{rep}"
        ld = rep == 0   # constants are loaded once and reused by later reps
        with (
            tc.tile_pool(name=f"acts{r}", bufs=1) as acts,
            tc.tile_pool(name=f"work{r}", bufs=2) as work,
            tc.tile_pool(name=f"ps{r}", bufs=2, space="PSUM") as ps,
            tc.tile_pool(name=f"dram{r}", bufs=1, space="DRAM") as dram,
        ):
            # ---- constants / weights (persistent across reps) ----
            if ld:
                consts["wq_sb"] = const.tile([P, HPC, KT, HD], bf16,
                                             name="wq_sbG")
                consts["wk_sb"] = const.tile([P, HPC, KT, HD], bf16,
                                             name="wk_sbG")
                consts["wv_sb"] = const.tile([P, KT, DC], bf16, name="wv_sbG")
                consts["cos_sb"] = const.tile([HD, S], bf16, name="cos_sbG")
                consts["sin_sb"] = const.tile([HD, S], bf16, name="sin_sbG")
                consts["msk_sb"] = const.tile([P, 896], bf16, name="msk_sbG")
                consts["ones_sb"] = const.tile([P, P], bf16, name="ones_sbG")
            wq_sb = consts["wq_sb"]
            wk_sb = consts["wk_sb"]
            wv_sb = consts["wv_sb"]
            cos_sb = consts["cos_sb"]
            sin_sb = consts["sin_sb"]
            msk_sb = consts["msk_sb"]
            ones_sb = consts["ones_sb"]

            # ---- persistent activations ----
            qfin = acts.tile([HD, HPC, S], bf16, name=f"qfin{r}")
            kfin = acts.tile([HD, HPC, S], bf16, name=f"kfin{r}")
            v_sb = acts.tile([P, NT, DC], bf16, name=f"v_sb{r}")
            # per-head attn.T so head-0's exchange never dep-couples to
            # head-1's writes
            attnT = [acts.tile([HD, S], bf16, name=f"attnT{h}{r}")
                     for h in range(HPC)]
            gp = nc.gpsimd
            import bass_rust as _br

            def dep(a, b, why="order"):
                _br.add_dep_helper(a.ins, b.ins, False, why)

            if RDMA:
                # receive buffers: ag[h][:, j, :] = head (2j+h) rows of
                # attn.T for my seq slice, written remotely by core j
                ag = [acts.tile([P, NCORES, SCS], bf16, name=f"ag{h}{r}")
                      for h in range(HPC)]
                if ld:
                    consts["myid_sb"] = const.tile([P, 1], u32,
                                                   name="myid_sbG")
                    consts["nbrs_sb"] = const.tile([P, NCORES], u32,
                                                   name="nbrs_sbG")
                myid_sb = consts["myid_sb"]
                nbrs_sb = consts["nbrs_sb"]

                # ---- topology-discovery prelude (overlaps projections) ----
                if ld:
                    nc.sync.dma_start(myid_sb[:], myid[:])
                for dl in range(NCORES):
                    gp.remote_dma_broadcast(
                        nbrs_sb[:, dl:dl + 1], myid_sb[:, 0:1],
                        remote_sem=nsem, local_sem=lsem, rdests=rdest(dl),
                    )
                trig_nbr = gp.trigger_dma(None)
                # The neighbor-id reg loads must (a) follow our own sends in
                # the Pool queue so the arrival wait can't deadlock, and (b)
                # wait for all 8 arrivals. The wait itself is attached
                # post-lowering (the tile scheduling pass cannot model
                # cross-core sem arrivals).
                mid = gp.partition_id()
                rblk = []
                for dl in range(NCORES):
                    reg = gp.alloc_register(f"rblk{dl}_{r}")
                    ld = gp.reg_load(reg, nbrs_sb[0:1, dl:dl + 1])
                    dep(ld, trig_nbr, "neighbor ids after own sends")
                    post_waits.append((ld.ins, nsem, 16 * (rep + 1)))
                    rblk.append(reg)
            else:
                # AllToAll bounce buffers, one per head so head-0's exchange
                # overlaps head-1's attention
                a2a_in = [dram.tile([NCORES, HD, SCS], bf16,
                                    name=f"a2ain{h}{r}") for h in range(HPC)]
                a2a_out = [dram.tile([NCORES, HD, SCS], bf16,
                                     name=f"a2aout{h}{r}") for h in range(HPC)]

            with tc.tile_pool(name=f"xtp{r}", bufs=1) as xtp:
                xt_sb = xtp.tile([P, KT, S], bf16, name=f"xt_sb{r}")
                qraw = xtp.tile([HD, HPC, S], bf16, name=f"qraw{r}")
                kraw = xtp.tile([HD, HPC, S], bf16, name=f"kraw{r}")
                # Load order: the qk0 phase is DMA-paced, so only what it
                # reads goes in front of the xt stream — a 2-ktile head-0
                # weight prefix unblocks the first matmuls at ~2us, the rest
                # of the head-0 halves follow xt0, and everything else
                # (wv, rope tables, head-1 halves, masks) trails the stream
                # in consumer order. All transfers serialize on the shared
                # DMA engines, so order is priority.
                if ld:
                    nc.sync.dma_start(wq_sb[:, 0, 0:2, :], wq[0, :, 0:2, :])
                    nc.sync.dma_start(wk_sb[:, 0, 0:2, :], wk[0, :, 0:2, :])
                nc.sync.dma_start(xt_sb[:, 0, :], xt_r[:, 0, :])
                if ld:
                    nc.sync.dma_start(wq_sb[:, 0, 2:KT, :], wq[0, :, 2:KT, :])
                    nc.sync.dma_start(wk_sb[:, 0, 2:KT, :], wk[0, :, 2:KT, :])
                for kt in range(1, KT):
                    nc.sync.dma_start(xt_sb[:, kt, :], xt_r[:, kt, :])
                if ld:
                    nc.sync.dma_start(wv_sb[:], wv_r)
                    nc.sync.dma_start(cos_sb[:], cost[:])
                    nc.sync.dma_start(sin_sb[:], sint[:])
                    nc.sync.dma_start(wq_sb[:, 1, :, :], wq[1, :, :, :])
                    nc.sync.dma_start(wk_sb[:, 1, :, :], wk[1, :, :, :])
                    nc.sync.dma_start(msk_sb[:], msk[:])
                    nc.sync.dma_start(ones_sb[:], ones[:])

                # 8 PSUM accumulation groups (one bank each) live at once:
                # (wq|wk) x 4 sq-blocks, contraction tiles streamed kt-outer
                grp8 = [("proj", 2), ("proj", 2), ("sc", 2), ("sc", 2),
                        ("attn2", 2), ("attn2", 2), ("attn", 1), ("l", 1)]

                def qk_proj(m):
                    pps = []
                    for g, (tag, b) in enumerate(grp8):
                        pps.append(ps.tile([P, SQW], fp32, tag=tag, bufs=b,
                                           name=f"pp{m}{g}"))
                    gs = [(w_sb, n) for w_sb in (wq_sb, wk_sb)
                          for n in range(NSQ)]
                    for kt in range(KT):
                        for g, (w_sb, n) in enumerate(gs):
                            nc.tensor.matmul(
                                pps[g][:],
                                w_sb[:, m, kt, :],
                                xt_sb[:, kt, n * SQW:(n + 1) * SQW],
                                start=(kt == 0),
                                stop=(kt == KT - 1),
                            )
                    for g, (w_sb, n) in enumerate(gs):
                        raw = qraw if w_sb is wq_sb else kraw
                        nc.scalar.copy(
                            raw[:, m, n * SQW:(n + 1) * SQW], pps[g][:]
                        )

                # rotate-half via SBUF partition-swap DMAs (sign is folded
                # into the host sin table; NCC_IBIR297 forbids DVE reading
                # two SBUF operands at different base partitions, so the
                # swap cannot ride the vector engine)
                qrot = xtp.tile([HD, HPC, S], bf16, name=f"qrot{r}")
                krot = xtp.tile([HD, HPC, S], bf16, name=f"krot{r}")

                def rope_rot(m, nsl):
                    for raw, rot in ((qraw, qrot), (kraw, krot)):
                        nc.sync.dma_start(rot[0:64, m, nsl],
                                          raw[64:128, m, nsl])
                        nc.sync.dma_start(rot[64:128, m, nsl],
                                          raw[0:64, m, nsl])

                def rope(m):
                    for raw, rot, fin in ((qraw, qrot, qfin),
                                          (kraw, krot, kfin)):
                        for n in range(NSQ):
                            nsl = slice(n * SQW, (n + 1) * SQW)
                            t1 = work.tile([P, SQW], fp32, tag="t1", bufs=2,
                                           name="t1")
                            t2 = work.tile([P, SQW], fp32, tag="t2", bufs=2,
                                           name="t2")
                            # t1 on GpSimd (otherwise idle) to shorten the
                            # per-slice DVE chain
                            nc.gpsimd.tensor_mul(t1[:], raw[:, m, nsl],
                                                 cos_sb[:, nsl])
                            nc.vector.tensor_mul(t2[:], rot[:, m, nsl],
                                                 sin_sb[:, nsl])
                            nc.vector.tensor_add(fin[:, m, nsl], t1[:], t2[:])

                def v_proj():
                    # copies on ACT (idle here) so DVE can run the rope muls;
                    # two m-tiles in flight so consecutive matmuls alternate
                    # PSUM banks (same-bank back-to-back stalls the PE)
                    for m0 in range(0, NT, 2):
                        pva = ps.tile([P, DC], fp32, tag="proj", bufs=2,
                                      name="pva")
                        pvb = ps.tile([P, DC], fp32, tag="proj", bufs=2,
                                      name="pvb")
                        for kt in range(KT):
                            for pv, m in ((pva, m0), (pvb, m0 + 1)):
                                nc.tensor.matmul(
                                    pv[:],
                                    xt_sb[:, kt, m * P:(m + 1) * P],
                                    wv_sb[:, kt, :],
                                    start=(kt == 0),
                                    stop=(kt == KT - 1),
                                )
                        nc.scalar.copy(v_sb[:, m0, :], pva[:])
                        nc.scalar.copy(v_sb[:, m0 + 1, :], pvb[:])

                def qk1_block(n):
                    # head-1 q/k projection for one 512-wide sq block:
                    # 2 PSUM groups so it can run concurrently with
                    # attention (which holds the other 4-6 banks)
                    nsl = slice(n * SQW, (n + 1) * SQW)
                    ppq = ps.tile([P, SQW], fp32, tag="proj", bufs=2,
                                  name="pp1q")
                    ppk = ps.tile([P, SQW], fp32, tag="proj", bufs=2,
                                  name="pp1k")
                    for kt in range(KT):
                        for w_sb, pp in ((wq_sb, ppq), (wk_sb, ppk)):
                            nc.tensor.matmul(
                                pp[:],
                                w_sb[:, 1, kt, :],
                                xt_sb[:, kt, nsl],
                                start=(kt == 0),
                                stop=(kt == KT - 1),
                            )
                    nc.scalar.copy(qraw[:, 1, nsl], ppq[:])
                    nc.scalar.copy(kraw[:, 1, nsl], ppk[:])

                def rope1_block(n):
                    nsl = slice(n * SQW, (n + 1) * SQW)
                    rope_rot(1, nsl)
                    for raw, rot, fin in ((qraw, qrot, qfin),
                                          (kraw, krot, kfin)):
                        t1 = work.tile([P, SQW], fp32, tag="t1", bufs=2,
                                       name="t1b")
                        t2 = work.tile([P, SQW], fp32, tag="t2", bufs=2,
                                       name="t2b")
                        # t1 on DVE: the Pool engine is occupied by the
                        # first AllToAll for its whole duration
                        nc.vector.tensor_mul(t1[:], raw[:, 1, nsl],
                                             cos_sb[:, nsl])
                        nc.vector.tensor_mul(t2[:], rot[:, 1, nsl],
                                             sin_sb[:, nsl])
                        nc.vector.tensor_add(fin[:, 1, nsl], t1[:], t2[:])

                st = {"mm": None, "exp": None}

                def attention_block(h, i):
                    hsl = slice(h * HD, (h + 1) * HD)
                    sq = slice(i * SQW, (i + 1) * SQW)
                    njt = 4 * i + 4
                    pa = ps.tile([HD, SQW], fp32, tag="attn", bufs=1,
                                 name="pa")
                    # softmax denominators accumulate on the PE: an
                    # all-ones stationary operand broadcasts the
                    # partition-sum of each exp tile into every row
                    pl = ps.tile([P, SQW], fp32, tag="l", bufs=1,
                                 name="pl")
                    pend = None   # unpaired full tile awaiting a partner
                    pend2 = None  # unpaired pair-sum awaiting a partner
                    first_pl = True
                    for j in range(njt):
                        # diagonal tiles (m>=1) are fully masked for
                        # columns < c0 = 128*m: narrow every stage to
                        # [c0:SQW] (m=0's triangle still needs full width)
                        m = j - 4 * i
                        c0 = 128 * m if m >= 1 else 0
                        csl = slice(c0, SQW)
                        sqn = slice(i * SQW + c0, (i + 1) * SQW)
                        psc = ps.tile([P, SQW], fp32,
                                      tag=("sc" if j % 2 else "attn2"),
                                      bufs=2, name="psc")
                        nc.tensor.matmul(
                            psc[:, csl],
                            kfin[:, h, j * P:(j + 1) * P],
                            qfin[:, h, sqn],
                            start=True, stop=True,
                        )
                        e = work.tile([P, SQW], bf16, tag="e", bufs=4,
                                      name="e")
                        st["exp"] = nc.scalar.activation(
                            e[:, csl], psc[:, csl], Exp, scale=SM)
                        if m >= 0:
                            em = work.tile([P, SQW], bf16, tag="em",
                                           bufs=4, name="em")
                            nc.vector.tensor_mul(
                                em[:, csl], e[:, csl],
                                msk_sb[:, 384: 896 - c0]
                            )
                            e = em
                        # denominator: pair adjacent unmasked exp tiles on
                        # DVE (bf16 fast mode) so the all-ones broadcast
                        # matmul runs once per pair; diagonal tiles keep
                        # their own (narrowed) matmul. PE is the pacer
                        # here, DVE has headroom. On the block's last tile
                        # the pl matmul goes first so the reciprocal runs
                        # under the final pa matmul instead of after it.
                        def pa_mm():
                            nc.tensor.matmul(
                                pa[:, csl],
                                v_sb[:, j, hsl],
                                e[:, csl],
                                start=(j == 0),
                                stop=(j == njt - 1),
                            )

                        if m < 0:
                            # full tiles: DVE-sum quads of exp tiles so the
                            # all-ones broadcast matmul runs once per four
                            pa_mm()
                            if pend is None:
                                pend = e
                                continue
                            es = work.tile([P, SQW], bf16, tag="es",
                                           bufs=2, name="es")
                            nc.vector.tensor_add(es[:], pend[:], e[:])
                            pend = None
                            if pend2 is None:
                                pend2 = es
                                continue
                            eq = work.tile([P, SQW], bf16, tag="eq",
                                           bufs=2, name="eq")
                            nc.vector.tensor_add(eq[:], pend2[:], es[:])
                            pend2 = None
                            st["mm"] = nc.tensor.matmul(
                                pl[:],
                                ones_sb[:],
                                eq[:],
                                start=first_pl,
                                stop=False,
                            )
                            first_pl = False
                            continue
                        if j != njt - 1:
                            pa_mm()
                        st["mm"] = nc.tensor.matmul(
                            pl[:, csl],
                            ones_sb[:],
                            e[:, csl],
                            start=first_pl,
                            stop=(j == njt - 1),
                        )
                        first_pl = False
                        if j == njt - 1:
                            pa_mm()
                    rec = work.tile([P, SQW], fp32, tag="rec", bufs=2,
                                    name="rec")
                    nc.vector.reciprocal(rec[:], pl[:])
                    nc.vector.tensor_mul(attnT[h][:, sq], pa[:], rec[:])
                    if not RDMA:
                        # ship finished 512-wide chunk into the AllToAll
                        # staging buffer (2 dest cores) as one permuted DMA
                        nc.sync.dma_start(
                            a2a_in[h][:].rearrange("j p s -> p j s")
                            [:, 2 * i:2 * i + 2, :],
                            attnT[h][:, i * SQW:(i + 1) * SQW],
                        )

                def exchange(h):
                    if RDMA:
                        a_base = attnT[h][:, 0:SCS]
                        o_base = ag[h][:, 0, :]
                        rslot = gp.alloc_register(f"rslot{h}{r}")
                        gp.reg_mul(rslot, mid, SCS)
                        gp.reg_add(rslot, rslot, int(o_base.offset))
                        for dl in range(NCORES):
                            roff = gp.alloc_register(f"roff{h}{dl}{r}")
                            gp.reg_mul(roff, rblk[dl], SCS)
                            gp.reg_add(roff, roff, int(a_base.offset))
                            gp.remote_dma_broadcast(
                                bass.AP(o_base.tensor, rslot, o_base.ap),
                                bass.AP(a_base.tensor, roff, a_base.ap),
                                remote_sem=rsem[h], local_sem=lsem,
                                rdests=rdest(dl),
                            )
                            gp.trigger_dma(None)
                    else:
                        nc.gpsimd.collective_compute(
                            "AllToAll",
                            mybir.AluOpType.bypass,
                            replica_groups=[list(range(NCORES))],
                            ins=[a2a_in[h][:].opt()],
                            outs=[a2a_out[h][:].opt()],
                        )

                # ---- phase order: finish head 0 as early as possible so
                # its exchange hides under the whole head-1 pipeline ----
                qk_proj(0)
                rope_rot(0, slice(0, S))
                rope(0)
                v_proj()
                for i in range(NSQ):
                    attention_block(0, i)
                exchange(0)
                # head-1: projection blocks software-pipelined one stage
                # ahead of their attention consumers
                qk1_block(0)
                rope1_block(0)
                qk1_block(1)
                rope1_block(1)
                attention_block(1, 0)
                qk1_block(2)
                rope1_block(2)
                attention_block(1, 1)
                qk1_block(3)
                rope1_block(3)
                attention_block(1, 2)
                attention_block(1, 3)
                exchange(1)
                last_attn_mm = st["mm"]
                last_exp = st["exp"]

            # gath pool opens in the SBUF space freed by xtp; the Wo.T load
            # rides the idle vector DGE queue (the sync queue carries the
            # exchange stores), head-0 k-tiles first so pass A can start
            # while the rest still streams
            with tc.tile_pool(name=f"gath{r}", bufs=1) as gath:
                wo_sb = gath.tile([P, KT, D], bf16, name=f"wo_sb{r}")
                ag_sb = None
                if not RDMA:
                    ag_sb = gath.tile([P, KT, SCS], bf16, name=f"ag_sb{r}")
                for h in range(HPC):
                    if not RDMA:
                        # gather-ins for this head (ACT queue, idle by now),
                        # gated on the head's AllToAll completion; one
                        # permuted DMA per head instead of 8 per-peer loads
                        nc.scalar.dma_start(
                            ag_sb[:].rearrange("p (k two) s -> p two k s",
                                               two=2)[:, h, :, :],
                            a2a_out[h][:].rearrange("j p s -> p j s"),
                        )
                    # this head's Wo.T k-tiles, one DMA each: small transfers
                    # keep the shared DMA engines preemptible so the last
                    # attnT staging stores (same resource) aren't delayed
                    for k in range(NCORES):
                        nc.sync.dma_start(wo_sb[:, 2 * k + h, :],
                                          wo_r[:, 2 * k + h, :])
                # ---- output projection (this core's 256 seq rows) ----
                # In rdma mode ag[h][:, j, :] was written directly into SBUF
                # by peer j; a PE-queue sem wait per head gates each pass.
                out_sb = gath.tile([P, SCS // P, D], fp32, name=f"out_sb{r}")
                grp_tags = [("proj", 2), ("proj", 2), ("sc", 2), ("sc", 2),
                            ("attn2", 2), ("attn2", 2), ("attn", 1), ("l", 1)]
                mns = [(m, n) for m in range(SCS // P) for n in range(NSQ)]
                po_tiles = []
                for g, (m, n) in enumerate(mns):
                    tag, b = grp_tags[g]
                    po_tiles.append(
                        ps.tile([P, SQW], fp32, tag=tag, bufs=b, name=f"po{g}")
                    )
                carrier = [None, None]
                for h in range(HPC):
                    for ki in range(NCORES):
                        for g, (m, n) in enumerate(mns):
                            kt = 2 * ki + h
                            stat = (ag[h][:, ki, m * P:(m + 1) * P] if RDMA
                                    else ag_sb[:, kt, m * P:(m + 1) * P])
                            mm = nc.tensor.matmul(
                                po_tiles[g][:],
                                stat,
                                wo_sb[:, kt, n * SQW:(n + 1) * SQW],
                                start=(h == 0 and ki == 0),
                                stop=(h == HPC - 1 and ki == NCORES - 1),
                                skip_group_check=True,
                            )
                            if carrier[h] is None:
                                # first pass-h matmul anchors the PE-queue
                                # order of the whole pass (and in rdma mode
                                # carries the post-lowering arrival wait)
                                carrier[h] = mm
                                if RDMA:
                                    post_waits.append(
                                        (mm.ins, rsem[h], 16 * (rep + 1)))
                                dep(mm, last_attn_mm if h == 0
                                    else carrier[0],
                                    f"wo pass-{h} anchor")
                            else:
                                dep(mm, carrier[h], f"wo pass-{h} order")
                # copy + store per group (quarter row) so the final out
                # DMA only trails the last PSUM copy by one 512-col chunk
                for g, (m, n) in enumerate(mns):
                    nc.vector.tensor_copy(
                        out_sb[:, m, n * SQW:(n + 1) * SQW], po_tiles[g][:]
                    )
                    nc.sync.dma_start(
                        out_r[m][:, n * SQW:(n + 1) * SQW],
                        out_sb[:, m, n * SQW:(n + 1) * SQW],
                    )

    with tile.TileContext(nc) as tc:
        with tc.tile_pool(name="constG", bufs=1) as constG:
            consts = {}
            for rep in range(reps):
                emit_body(tc, rep, constG, consts)

    # attach cross-core arrival waits; nc.compile()'s
    # generate_event_semaphores pass legalizes multi-wait instructions
    for inst, sem, val in post_waits:
        w = mybir.SyncWait(sync_type="semaphore", id=sem.num,
                           wait_mode="sem-ge-imm", wait_value=val,
                           ant_name=sem.name)
        si = inst.sync_info
        if si is None:
            inst.sync_info = mybir.SyncInfo(on_wait=[w], on_update=[])
        else:
            inst.sync_info = mybir.SyncInfo(
                on_wait=list(si.on_wait) + [w],
                on_update=list(si.on_update))

    nc.compile()
    return nc


def _get_nc(reps=1):
    key = ("nc", reps)
    if key not in _NC_CACHE:
        _NC_CACHE[key] = _build_nc(reps)
    return _NC_CACHE[key]


def _host_tables():
    import ml_dtypes

    bf = ml_dtypes.bfloat16
    inv_freq = 1.0 / (10000.0 ** (np.arange(0, HD, 2, dtype=np.float32) / HD))
    t = np.arange(S, dtype=np.float32)
    freqs = np.outer(t, inv_freq)
    emb = np.concatenate([freqs, freqs], axis=-1)        # [S, HD]
    cosT = np.ascontiguousarray(np.cos(emb).T)
    sinT = np.ascontiguousarray(np.sin(emb).T)
    # rotate-half is a pure partition swap on-device; its sign lives here:
    # fin[p] = raw[p]*cos[p] + raw[(p+64)%128]*sinT'[p], sinT' negated for
    # the first half of the head dim
    sinT[0:64] *= -1.0

    y = np.arange(896)[None, :]
    tl = np.arange(P)[:, None]
    mskM = (tl <= (y - 384)).astype(np.float32)

    ones = np.ones((P, P), dtype=np.float32)
    return (cosT.astype(bf), sinT.astype(bf),
            mskM.astype(bf), ones.astype(bf))


def _prep_in_maps(hidden_states, Wq, Wk, Wv, Wo):
    import ml_dtypes

    bf = ml_dtypes.bfloat16
    X = np.asarray(hidden_states, dtype=np.float32).reshape(S, D)
    Wq = np.asarray(Wq, dtype=np.float32)
    Wk = np.asarray(Wk, dtype=np.float32)
    Wv = np.asarray(Wv, dtype=np.float32)
    Wo = np.asarray(Wo, dtype=np.float32)

    XT = np.ascontiguousarray(X.T).astype(bf)
    WoT = np.ascontiguousarray(Wo.T).astype(bf)
    cosT, sinT, mskM, ones_ = _host_tables()

    def _wh(Wc):
        # [DC, D] head-rows slice -> head-major [HPC, P, KT, HD] with the
        # (kt, hd) plane contiguous per partition so each head half streams
        # as one large-descriptor DMA
        WT = np.ascontiguousarray(Wc.T)          # [D, DC]
        return np.ascontiguousarray(
            WT.reshape(KT, P, HPC, HD).transpose(2, 1, 0, 3)
        ).astype(bf)

    in_maps = []
    for c in range(NCORES):
        sl = slice(DC * c, DC * (c + 1))
        in_maps.append({
            "myid": np.full((P, 1), c, dtype=np.uint32),
            "xt": XT,
            "wq": _wh(Wq[sl]),
            "wk": _wh(Wk[sl]),
            "wv": np.ascontiguousarray(Wv[sl].T).astype(bf),
            "wo": WoT,
            "cost": cosT,
            "sint": sinT,
            "msk": mskM,
            "ones": ones_,
        })
    return in_maps


def kernel(hidden_states, Wq, Wk, Wv, Wo):
    global LAST_RESULTS
    from concourse.bass_utils import run_bass_kernel_spmd

    in_maps = _prep_in_maps(hidden_states, Wq, Wk, Wv, Wo)
    nc = _get_nc()
    res = run_bass_kernel_spmd(nc, in_maps, core_ids=list(range(NCORES)))
    LAST_RESULTS = res

    out = np.concatenate(
        [np.asarray(res.results[c]["out"]) for c in range(NCORES)], axis=0
    )
    return out.reshape(1, S, D).astype(np.float32)

